# revision 2
# baseline (speedup 1.0000x reference)
"""Trainium2 Bass kernel for nn_CombinedLoss (regression MSE + masked binary focal loss).

Data-parallel over 8 NeuronCores: each core reduces its batch shard to
per-class partial sums; the final (tiny) weighted combination happens on host
in float64.

Math (per element of the 13 presence classes, t in {-1, 0, 1}):
    z  = (1 - 2t) * x          (so z = x for t=0, -x for t=1)
    focal(x, t) = softplus(z) * sigmoid(z)^2      for valid t (t != -1)
    weighted by w_c if t==1 else (1-w_c), masked out for t==-1.

On-device we avoid masking entirely by reducing three per-class sums over ALL
elements (including t==-1 garbage, which is finite):
    S0_c  = sum f          Sh_c  = sum f*t        Sh2_c = sum f*t^2
Host side:   F1 = (Sh+Sh2)/2  (t==1 sum),  F0 = S0-Sh2  (t==0 sum)
    focal_total = sum_c (1-w_c)*F0_c + w_c*F1_c

softplus/sigmoid use only the `natural_log_exp_and_others` ACT table set
(softplus has no HW table on this toolchain):
    e  = exp(z)            [ACT Exp,   scale=-2 applied to z' = (t-.5)*x]
    sp = ln(e + 1)         [ACT Ln,    free bias=+1]     == softplus(z)
    s2 = exp(2*(z - sp))   [ACT Exp,   scale=+2]         == sigmoid(z)^2
so the ACT engine never switches table sets.

The per-class reductions run on the (otherwise idle) TensorEngine as
ones-vector matmuls accumulating into PSUM in fp32.
"""

import sys

if "/opt/trn_rl_repo" not in sys.path:
    sys.path.insert(0, "/opt/trn_rl_repo")

import numpy as np

NCORES = 8
B = 2_097_152
BS = B // NCORES          # 262144 rows per core
P = 128                   # SBUF partitions
RPP = BS // P             # 2048 rows per partition
T = 64                    # rows per tile
NT = RPP // T             # 32 tiles
G = 2                     # tiles per group (one elementwise op instr per group)
NGROUP = NT // G          # 16 groups
FD_FULL = T * 16          # 1024 fp32 per partition per tile (4KB DMA run)
FD_C = T * 13             # 832 class elements per partition per tile
FD_R = T * 3              # 192 regression elements per partition per tile
NPART = 3 * FD_C + FD_R   # 2688 partial-sum cells per core


def build(reps: int = 1):
    import concourse.bacc as bacc
    import concourse.mybir as mybir
    import concourse.tile as tile

    dt = mybir.dt
    AF = mybir.ActivationFunctionType
    OP = mybir.AluOpType

    nc = bacc.Bacc("TRN2", target_bir_lowering=False, debug=False,
                   num_devices=NCORES)
    x_d = nc.dram_tensor("output", [BS, 16], dt.float32, kind="ExternalInput")
    t_d = nc.dram_tensor("target", [BS, 16], dt.float32, kind="ExternalInput")
    po_d = nc.dram_tensor("partials", [1, NPART], dt.float32,
                          kind="ExternalOutput")

    # [128, 32768] per-partition contiguous row blocks
    xv = x_d.ap().rearrange("(p r) c -> p (r c)", p=P)
    tv = t_d.ap().rearrange("(p r) c -> p (r c)", p=P)

    FDGF = G * FD_FULL
    FDGC = G * FD_C
    FDGR = G * FD_R

    with tile.TileContext(nc) as tc:
        with (
            tc.tile_pool(name="io", bufs=2) as io_pool,
            tc.tile_pool(name="f32", bufs=2) as f32_pool,
            tc.tile_pool(name="b16", bufs=2) as b16_pool,
            tc.tile_pool(name="cst", bufs=1) as cst_pool,
            tc.tile_pool(name="acc", bufs=1, space="PSUM") as psum_pool,
        ):
            ones = cst_pool.tile([P, 1], dt.bfloat16, tag="ones")
            nc.vector.memset(ones[:], 1.0)

            p0 = psum_pool.tile([1, FD_C], dt.float32, tag="p0")
            p1 = psum_pool.tile([1, FD_C], dt.float32, tag="p1")
            p2 = psum_pool.tile([1, FD_C], dt.float32, tag="p2")
            pq = psum_pool.tile([1, FD_R], dt.float32, tag="pq")

            for rep in range(reps):
                for g in range(NGROUP):
                    xg = io_pool.tile([P, FDGF], dt.float32, tag="xg")
                    tg = io_pool.tile([P, FDGF], dt.float32, tag="tg")
                    for i in range(G):
                        j = g * G + i
                        sl_s = slice(i * FD_FULL, (i + 1) * FD_FULL)
                        sl_d = slice(j * FD_FULL, (j + 1) * FD_FULL)
                        nc.sync.dma_start(xg[:, sl_s], xv[:, sl_d])
                        nc.sync.dma_start(tg[:, sl_s], tv[:, sl_d])

                    x3 = xg[:].rearrange("p (r c) -> p r c", c=16)
                    t3 = tg[:].rearrange("p (r c) -> p r c", c=16)
                    xc, tc_v = x3[:, :, 3:16], t3[:, :, 3:16]
                    xr, tr_v = x3[:, :, 0:3], t3[:, :, 0:3]

                    # a = 1 - 2t (exact small ints; fp32 tensor_scalar is 2x)
                    ag = f32_pool.tile([P, FDGC], dt.float32, tag="a")
                    nc.vector.tensor_scalar(
                        ag[:], tc_v, -2.0, 1.0, OP.mult, OP.add)
                    # z = x * a = (1-2t) x   (on GPSIMD, plain fp32 multiply)
                    zg = f32_pool.tile([P, FDGC], dt.float32, tag="z")
                    nc.gpsimd.tensor_tensor(zg[:], xc, ag[:], OP.mult)

                    # packed bf16 copy of class targets (exact for -1/0/1)
                    tpg = b16_pool.tile([P, FDGC], dt.bfloat16, tag="tp")
                    nc.gpsimd.tensor_copy(tpg[:], tc_v)

                    # regression: q = (x - t)^2 in bf16
                    dg = b16_pool.tile([P, FDGR], dt.bfloat16, tag="d")
                    nc.vector.tensor_tensor(dg[:], xr, tr_v, OP.subtract)
                    qg = b16_pool.tile([P, FDGR], dt.bfloat16, tag="q")
                    nc.vector.tensor_tensor(qg[:], dg[:], dg[:], OP.mult)

                    # e = exp(z)
                    eg = f32_pool.tile([P, FDGC], dt.float32, tag="scr")
                    nc.scalar.activation(eg[:], zg[:], AF.Exp)
                    # sp = ln(e + 1) = softplus(z), rounded to bf16
                    spg = b16_pool.tile([P, FDGC], dt.bfloat16, tag="sp")
                    nc.scalar.activation(spg[:], eg[:], AF.Ln, bias=1.0)
                    # v = z - sp  (mixed fp32/bf16)
                    vg = f32_pool.tile([P, FDGC], dt.float32, tag="scr")
                    nc.vector.tensor_tensor(vg[:], zg[:], spg[:], OP.subtract)
                    # s2 = exp(2 v) = sigmoid(z)^2
                    s2g = b16_pool.tile([P, FDGC], dt.bfloat16, tag="s2")
                    nc.scalar.activation(s2g[:], vg[:], AF.Exp, scale=2.0)

                    fg = b16_pool.tile([P, FDGC], dt.bfloat16, tag="f")
                    nc.vector.tensor_tensor(fg[:], spg[:], s2g[:], OP.mult)
                    hg = b16_pool.tile([P, FDGC], dt.bfloat16, tag="h")
                    nc.vector.tensor_tensor(hg[:], fg[:], tpg[:], OP.mult)
                    h2g = b16_pool.tile([P, FDGC], dt.bfloat16, tag="h2")
                    nc.vector.tensor_tensor(h2g[:], hg[:], tpg[:], OP.mult)

                    for i in range(G):
                        j = g * G + i
                        st = j == 0
                        fin = j == NT - 1
                        off = i * FD_C
                        for (acc, src) in ((p0, fg), (p1, hg), (p2, h2g)):
                            nc.tensor.matmul(acc[:, 0:512], ones[:],
                                             src[:, off:off + 512],
                                             start=st, stop=fin)
                            nc.tensor.matmul(acc[:, 512:FD_C], ones[:],
                                             src[:, off + 512:off + FD_C],
                                             start=st, stop=fin)
                        nc.tensor.matmul(pq[:], ones[:],
                                         qg[:, i * FD_R:(i + 1) * FD_R],
                                         start=st, stop=fin)

            outt = cst_pool.tile([1, NPART], dt.float32, tag="out")
            nc.scalar.copy(outt[:, 0:FD_C], p0[:])
            nc.scalar.copy(outt[:, FD_C:2 * FD_C], p1[:])
            nc.scalar.copy(outt[:, 2 * FD_C:3 * FD_C], p2[:])
            nc.scalar.copy(outt[:, 3 * FD_C:NPART], pq[:])
            nc.sync.dma_start(po_d.ap(), outt[:])

    nc.compile()
    return nc


# ---------------------------------------------------------------------------
# Cached PJRT executor (jit once per process; later calls are cheap).
# Mirrors concourse.bass2jax.run_bass_via_pjrt for the 8-core SPMD case.
# ---------------------------------------------------------------------------

_EXEC = None


def _get_executor():
    global _EXEC
    if _EXEC is not None:
        return _EXEC

    import jax
    import concourse.mybir as mybir
    from concourse import bass2jax
    from jax.sharding import Mesh, PartitionSpec
    from jax.experimental.shard_map import shard_map

    nc = build(1)
    bass2jax.install_neuronx_cc_hook()

    partition_name = (nc.partition_id_tensor.name
                      if nc.partition_id_tensor else None)
    in_names, out_names, out_avals = [], [], []
    for alloc in nc.m.functions[0].allocations:
        if not isinstance(alloc, mybir.MemoryLocationSet):
            continue
        name = alloc.memorylocations[0].name
        if alloc.kind == "ExternalInput":
            if name != partition_name:
                in_names.append(name)
        elif alloc.kind == "ExternalOutput":
            out_names.append(name)
            out_avals.append(jax.core.ShapedArray(
                tuple(alloc.tensor_shape), mybir.dt.np(alloc.dtype)))

    n_params = len(in_names)
    n_outs = len(out_avals)
    all_in_names = list(in_names) + list(out_names)
    if partition_name is not None:
        all_in_names.append(partition_name)

    def _body(*args):
        operands = list(args)
        if partition_name is not None:
            operands.append(bass2jax.partition_id_tensor())
        return tuple(bass2jax._bass_exec_p.bind(
            *operands,
            out_avals=tuple(out_avals),
            in_names=tuple(all_in_names),
            out_names=tuple(out_names),
            lowering_input_output_aliases=(),
            sim_require_finite=True,
            sim_require_nnan=True,
            nc=nc,
        ))

    devices = jax.devices()[:NCORES]
    mesh = Mesh(np.asarray(devices), ("core",))
    in_specs = (PartitionSpec("core"),) * (n_params + n_outs)
    out_specs = (PartitionSpec("core"),) * n_outs
    donate = tuple(range(n_params, n_params + n_outs))
    sharded = jax.jit(
        shard_map(_body, mesh=mesh, in_specs=in_specs, out_specs=out_specs,
                  check_rep=False),
        donate_argnums=donate, keep_unused=True)

    _EXEC = (sharded, in_names, out_names, out_avals)
    return _EXEC


def run_device_partials(output: np.ndarray, target: np.ndarray) -> np.ndarray:
    """Run the SPMD kernel; returns per-core partials [NCORES, NPART] fp32."""
    sharded, in_names, out_names, out_avals = _get_executor()
    feeds = {"output": np.ascontiguousarray(output, dtype=np.float32),
             "target": np.ascontiguousarray(target, dtype=np.float32)}
    ins = [feeds[n] for n in in_names]
    zeros = [np.zeros((NCORES * a.shape[0],) + a.shape[1:], a.dtype)
             for a in out_avals]
    outs = sharded(*ins, *zeros)
    idx = out_names.index("partials")
    return np.asarray(outs[idx]).reshape(NCORES, NPART)


def combine_partials(partials: np.ndarray,
                     binary_class_weights: np.ndarray) -> np.float32:
    """Host-side fp64 combination of per-core partial sums into the loss."""
    p = partials.astype(np.float64).sum(axis=0)
    S0 = p[0:FD_C].reshape(T, 13).sum(axis=0)
    Sh = p[FD_C:2 * FD_C].reshape(T, 13).sum(axis=0)
    Sh2 = p[2 * FD_C:3 * FD_C].reshape(T, 13).sum(axis=0)
    Q = p[3 * FD_C:NPART].reshape(T, 3).sum(axis=0)
    w = np.asarray(binary_class_weights, dtype=np.float64)
    F1 = (Sh + Sh2) / 2.0
    F0 = S0 - Sh2
    focal = np.sum((1.0 - w) * F0 + w * F1)
    mse = Q / float(B)
    loss = 10.0 * mse[0] + mse[1] + mse[2] + focal
    return np.float32(loss)


def kernel(output: np.ndarray, target: np.ndarray,
           binary_class_weights: np.ndarray) -> np.ndarray:
    partials = run_device_partials(output, target)
    return np.asarray(combine_partials(partials, binary_class_weights))
